# revision 12
# baseline (speedup 1.0000x reference)
"""Trainium2 Bass kernel for nn_Aggregator (GNN relational message passing).

Computes: out[h] = sum_{e: head_e==h} all_emb[tail_e] * weight[type_e] * aug_e

Strategy (8 NeuronCores, SPMD):
  - Shard output nodes (head ranges of 12500) across the 8 cores; each core
    gets exactly the edges whose head falls in its range (host bins them).
    No collective needed - host concatenates the 8 output shards.
  - Host sorts edges by head into 128-node output tiles. Within a tile,
    edges are grouped by tail window (4 windows of 25000 source rows, since
    dma_gather indices are int16) and each (tile, window) run is padded to a
    fixed chunk capacity so the device schedule is fully static.
  - Gather: one dma_gather per (7-tile group, window) pulls all_emb rows
    (256B each) into SBUF, window-major. Processing is tile-major and reads
    the gather buffer through a strided AP.
  - Per 128-edge chunk:
      Wsel = onehotAugT^T @ W4stack on PE      -> aug_e*weight[type_e] [e,64]
      S    = is_equal(iota, head_local) on DVE -> head one-hot (fp16) [e,128]
      V    = G * Wsel on DVE (fp16 out)                              [e,64]
      acc(psum f32) += S^T @ V on PE   (accumulated per node tile)
    Tile flush: PSUM -> SBUF staging on ScalarE; one output DMA at the end.
"""

import os

import numpy as np

import concourse.bacc as bacc
import concourse.tile as tile
from concourse import bass, mybir
from concourse.bass_utils import run_bass_kernel_spmd

P = 128
C = 64  # channels
R = 24  # relations
RP = 32  # relations padded (4 chunks of 32 rows stack into 128 partitions)
N_NODES = 100000
N_CORES = 8
NPC = N_NODES // N_CORES  # 12500 nodes per core
TILES = (NPC + P - 1) // P  # 98 output tiles per core
WINS = 4
WSZ = 25000  # window size (int16 gather index reach)
GROUP_T = 7  # tiles per gather group (98 = 14 * 7)
NGRP = TILES // GROUP_T

_NC_CACHE = {}


def _ap3(ap, offset_elems, mid_step, mid_n, inner_n):
    """[128, mid_n, inner_n] view of a 2D SBUF tile AP with custom strides."""
    return bass.AP(
        ap.tensor,
        ap.offset + offset_elems,
        [list(ap.ap[0]), [mid_step, mid_n], [1, inner_n]],
    )


def _build(capw: int, ngrp: int = NGRP):
    """Build the Bass module for per-(tile,window) chunk capacity capw."""
    cap = WINS * capw  # chunks per tile
    nchunk = TILES * cap
    gc = GROUP_T * cap  # chunks per gather group
    ncol = (nchunk // 4) * P  # onehot columns

    nc = bacc.Bacc("TRN2", target_bir_lowering=False)

    f32 = mybir.dt.float32
    f16 = mybir.dt.float16
    i16 = mybir.dt.int16

    emb_d = nc.dram_tensor("all_emb", [N_NODES, C], f32, kind="ExternalInput")
    idx_d = nc.dram_tensor("idx16", [P, nchunk * 8], i16, kind="ExternalInput")
    head_d = nc.dram_tensor("head_local", [P, nchunk], f16,
                            kind="ExternalInput")
    oh_d = nc.dram_tensor("oh", [P, ncol], f32, kind="ExternalInput")
    w4_d = nc.dram_tensor("w4", [P, 4 * C], f32, kind="ExternalInput")
    iota_d = nc.dram_tensor("iota", [P, P], f16, kind="ExternalInput")
    out_d = nc.dram_tensor("out", [TILES * P, C], f32, kind="ExternalOutput")

    with tile.TileContext(nc) as tc:
        with (
            tc.tile_pool(name="res", bufs=1) as res,
            tc.tile_pool(name="gp", bufs=2) as gp,
            tc.tile_pool(name="ohp", bufs=2) as ohp,
            tc.tile_pool(name="sp", bufs=2) as sp,
            tc.tile_pool(name="vp", bufs=2) as vp,
            tc.tile_pool(name="wselp", bufs=2, space="PSUM") as wselp,
            tc.tile_pool(name="accp", bufs=2, space="PSUM") as accp,
        ):
            idx_t = res.tile([P, nchunk * 8], i16)
            head_t = res.tile([P, nchunk], f16)
            iota_t = res.tile([P, P], f16)
            w4_t = res.tile([P, 4 * C], f32)
            ostage = res.tile([P, TILES * C], f32)

            nc.sync.dma_start(idx_t[:], idx_d[:])
            nc.sync.dma_start(head_t[:], head_d[:])
            nc.sync.dma_start(iota_t[:], iota_d[:])
            nc.sync.dma_start(w4_t[:], w4_d[:])

            for g in range(ngrp):
                # gather the whole group, one call per source window
                g_t = gp.tile([P, gc * C], f32, tag="g")
                for w in range(WINS):
                    nidx = GROUP_T * capw * P
                    s0 = g * gc + GROUP_T * capw * w  # first gather slot
                    nc.gpsimd.dma_gather(
                        out_ap=g_t[
                            :, GROUP_T * capw * w * C : GROUP_T * capw * (w + 1) * C
                        ].rearrange("p (j c) -> p j c", c=C),
                        in_ap=emb_d[w * WSZ : min((w + 1) * WSZ, N_NODES), :],
                        idxs_ap=idx_t[:, s0 * 8 : s0 * 8 + nidx // 16],
                        num_idxs=nidx,
                        num_idxs_reg=nidx,
                        elem_size=C,
                        single_packet=False,
                    )

                # onehot slab for the group's chunks
                oh_t = ohp.tile([P, (gc // 4) * P], f32, tag="oh")
                nc.sync.dma_start(
                    oh_t[:], oh_d[:, (g * gc // 4) * P : ((g + 1) * gc // 4) * P]
                )

                for tloc in range(GROUP_T):
                    t = g * GROUP_T + tloc
                    j0 = t * cap  # first processing chunk of the tile

                    # head one-hots for all chunks of the tile (fp16, 2x DVE)
                    s_t = sp.tile([P, cap * P], f16, tag="s")
                    nc.vector.tensor_tensor(
                        out=s_t[:].rearrange("p (q n) -> p q n", n=P),
                        in0=head_t[:, j0 : j0 + cap].unsqueeze(2).to_broadcast(
                            [P, cap, P]
                        ),
                        in1=iota_t[:].unsqueeze(1).to_broadcast([P, cap, P]),
                        op=mybir.AluOpType.is_equal,
                    )

                    # Wsel for all chunks: one stacked matmul per 4 chunks
                    wsel_t = wselp.tile([P, cap * C], f32, tag="wsel")
                    for b in range(cap // 4):
                        gcol = ((j0 + 4 * b) // 4) * P - (g * gc // 4) * P
                        nc.tensor.matmul(
                            out=wsel_t[:, 4 * b * C : 4 * (b + 1) * C],
                            lhsT=oh_t[:, gcol : gcol + P],
                            rhs=w4_t[:],
                            start=True,
                            stop=True,
                        )

                    # V = G * Wsel for the whole tile in one DVE op.
                    # G slots for (w, kw) sit at group offset
                    # 7*capw*w + tloc*capw + kw -> strided 3D view.
                    v_t = vp.tile([P, cap * C], f16, tag="v")
                    g_view = _ap3(
                        g_t[:],
                        tloc * capw * C,
                        GROUP_T * capw * C,
                        WINS,
                        capw * C,
                    )
                    nc.vector.tensor_tensor(
                        out=v_t[:].rearrange("p (w x) -> p w x", w=WINS),
                        in0=g_view,
                        in1=wsel_t[:].rearrange("p (w x) -> p w x", w=WINS),
                        op=mybir.AluOpType.mult,
                    )

                    # scatter-accumulate the tile's chunks into PSUM
                    acc_t = accp.tile([P, C], f32, tag="acc")
                    for q in range(cap):
                        nc.tensor.matmul(
                            out=acc_t[:],
                            lhsT=s_t[:, q * P : (q + 1) * P],
                            rhs=v_t[:, q * C : (q + 1) * C],
                            start=(q == 0),
                            stop=(q == cap - 1),
                        )

                    nc.scalar.copy(
                        out=ostage[:, t * C : (t + 1) * C], in_=acc_t[:]
                    )

            nc.sync.dma_start(
                out_d[:].rearrange("(t p) c -> p t c", p=P),
                ostage[:].rearrange("p (t c) -> p t c", c=C),
            )

    nc.finalize()
    return nc


def _prep(all_emb, edge_index, edge_type, weight, aug_edge_weight):
    """Host-side binning/padding. Returns (capw, in_maps)."""
    head = np.asarray(edge_index[0], dtype=np.int64)
    tail = np.asarray(edge_index[1], dtype=np.int64)
    etype = np.asarray(edge_type, dtype=np.int64)
    aug = np.asarray(aug_edge_weight, dtype=np.float32).reshape(-1)
    emb = np.ascontiguousarray(np.asarray(all_emb, dtype=np.float32))
    w = np.asarray(weight, dtype=np.float32)

    order = np.argsort(head, kind="stable")
    h_s = head[order]
    bounds = np.searchsorted(h_s, np.arange(N_CORES + 1) * NPC)

    capw = 1
    per_core = []
    for c_i in range(N_CORES):
        e_idx = order[bounds[c_i] : bounds[c_i + 1]]
        h_loc = h_s[bounds[c_i] : bounds[c_i + 1]] - c_i * NPC
        t_loc = tail[e_idx]
        tw = (h_loc >> 7) * WINS + t_loc // WSZ  # (tile, window) bucket
        cnt = np.bincount(tw, minlength=TILES * WINS)
        capw = max(capw, int(-(-cnt.max() // P)))
        per_core.append((e_idx, h_loc, t_loc, tw, cnt))

    cap = WINS * capw
    nchunk = TILES * cap
    gc = GROUP_T * cap
    ncol = (nchunk // 4) * P

    iota = np.tile(np.arange(P, dtype=np.float16), (P, 1))
    w4 = np.zeros((P, 4 * C), dtype=np.float32)
    for s in range(4):
        w4[RP * s : RP * s + R, s * C : (s + 1) * C] = w

    in_maps = []
    for c_i in range(N_CORES):
        e_idx, h_loc, t_loc, tw, cnt = per_core[c_i]
        o2 = np.argsort(tw, kind="stable")  # group edges by (tile, window)
        e_idx, h_loc, t_loc, tw = e_idx[o2], h_loc[o2], t_loc[o2], tw[o2]
        starts = np.cumsum(cnt) - cnt
        pos = np.arange(len(e_idx)) - starts[tw]

        tile_id = tw // WINS
        win = tw % WINS
        kw = pos >> 7
        p = pos & (P - 1)

        # processing chunk id (tile-major)
        j = tile_id * cap + win * capw + kw
        # gather slot (window-major within each 7-tile group)
        grp = tile_id // GROUP_T
        tloc = tile_id % GROUP_T
        slot = grp * gc + GROUP_T * capw * win + tloc * capw + kw
        gi = slot * P + p  # flat gather index

        idx16 = np.zeros((P, nchunk * 8), np.int16)
        val16 = (t_loc - win * WSZ).astype(np.int16)
        rows = (gi % 16).astype(np.int64)
        cols = (gi // 16).astype(np.int64)
        for rep in range(8):
            idx16[rep * 16 + rows, cols] = val16

        head_a = np.zeros((P, nchunk), dtype=np.float16)
        head_a[p, j] = (h_loc - (tile_id << 7)).astype(np.float16)

        oh = np.zeros((P, ncol), dtype=np.float32)
        q_r = RP * (j & 3) + etype[e_idx]
        col = (j >> 2) * P + p
        oh[q_r, col] = aug[e_idx]

        in_maps.append(
            {
                "all_emb": emb,
                "idx16": idx16,
                "head_local": head_a,
                "oh": oh,
                "w4": w4,
                "iota": iota,
            }
        )
    return capw, in_maps


def kernel(all_emb, edge_index, edge_type, weight, aug_edge_weight):
    capw, in_maps = _prep(all_emb, edge_index, edge_type, weight,
                          aug_edge_weight)
    if capw not in _NC_CACHE:
        _NC_CACHE[capw] = _build(capw)
    nc = _NC_CACHE[capw]

    trace = bool(int(os.environ.get("KERNEL_TRACE", "0")))
    res = run_bass_kernel_spmd(
        nc,
        in_maps,
        core_ids=list(range(N_CORES)),
        trace=trace,
    )
    kernel._last_results = res
    out = np.concatenate(
        [res.results[c_i]["out"][:NPC] for c_i in range(N_CORES)], axis=0
    )
    return out


# revision 14
# speedup vs baseline: 66.7263x; 66.7263x over previous
"""Trainium2 Bass kernel for nn_Aggregator (GNN relational message passing).

Computes: out[h] = sum_{e: head_e==h} all_emb[tail_e] * weight[type_e] * aug_e

Strategy (8 NeuronCores, SPMD):
  - Shard output nodes (head ranges of 12500) across the 8 cores; each core
    gets exactly the edges whose head falls in its range (host bins them).
    No collective needed - host concatenates the 8 output shards.
  - Host sorts edges by head into 128-node output tiles. Within a tile,
    edges are grouped by tail window (4 windows of 25000 source rows, since
    dma_gather indices are int16) and each (tile, window) run is padded to a
    fixed chunk capacity so the device schedule is fully static.
  - Gather: one dma_gather per (7-tile group, window) pulls all_emb rows
    (256B each) into SBUF, window-major. Processing is tile-major and reads
    the gather buffer through a strided AP.
  - Per 128-edge chunk:
      Wsel = onehotAugT^T @ W4stack on PE      -> aug_e*weight[type_e] [e,64]
      S    = is_equal(iota, head_local) on DVE -> head one-hot (fp16) [e,128]
      V    = G * Wsel on DVE (fp16 out)                              [e,64]
      acc(psum f32) += S^T @ V on PE   (accumulated per node tile)
    Tile flush: PSUM -> SBUF staging on ScalarE; one output DMA at the end.
"""

import os

import numpy as np

import concourse.bacc as bacc
import concourse.tile as tile
from concourse import bass, mybir
from concourse.bass_utils import run_bass_kernel_spmd

P = 128
C = 64  # channels
R = 24  # relations
RP = 32  # relations padded (4 chunks of 32 rows stack into 128 partitions)
N_NODES = 100000
N_CORES = 8
NPC = N_NODES // N_CORES  # 12500 nodes per core
TILES = (NPC + P - 1) // P  # 98 output tiles per core
WINS = 4
WSZ = 25000  # window size (int16 gather index reach)
GROUP_T = 7  # tiles per gather group (98 = 14 * 7)
NGRP = TILES // GROUP_T

_NC_CACHE = {}


def _ap3(ap, offset_elems, mid_step, mid_n, inner_n):
    """[128, mid_n, inner_n] view of a 2D SBUF tile AP with custom strides."""
    return bass.AP(
        ap.tensor,
        ap.offset + offset_elems,
        [list(ap.ap[0]), [mid_step, mid_n], [1, inner_n]],
    )


def _build(capw: int, ngrp: int = NGRP, bench_loop: int = 0):
    """Build the Bass module for per-(tile,window) chunk capacity capw.

    bench_loop > 0 wraps the whole compute schedule in a device-side For_i
    that repeats it that many times (benchmark amplification only).
    """
    cap = WINS * capw  # chunks per tile
    nchunk = TILES * cap
    gc = GROUP_T * cap  # chunks per gather group
    ncol = (nchunk // 4) * P  # onehot columns

    nc = bacc.Bacc("TRN2", target_bir_lowering=False)

    f32 = mybir.dt.float32
    f16 = mybir.dt.float16
    i16 = mybir.dt.int16

    emb_d = nc.dram_tensor("all_emb", [N_NODES, C], f32, kind="ExternalInput")
    idx_d = nc.dram_tensor("idx16", [P, nchunk * 8], i16, kind="ExternalInput")
    head_d = nc.dram_tensor("head_local", [P, nchunk], f16,
                            kind="ExternalInput")
    oh_d = nc.dram_tensor("oh", [P, ncol], f32, kind="ExternalInput")
    w4_d = nc.dram_tensor("w4", [P, 4 * C], f32, kind="ExternalInput")
    iota_d = nc.dram_tensor("iota", [P, P], f16, kind="ExternalInput")
    out_d = nc.dram_tensor("out", [TILES * P, C], f32, kind="ExternalOutput")

    with tile.TileContext(nc) as tc:
        with (
            tc.tile_pool(name="res", bufs=1) as res,
            tc.tile_pool(name="gp", bufs=2) as gp,
            tc.tile_pool(name="ohp", bufs=2) as ohp,
            tc.tile_pool(name="sp", bufs=2) as sp,
            tc.tile_pool(name="vp", bufs=2) as vp,
            tc.tile_pool(name="wselp", bufs=2, space="PSUM") as wselp,
            tc.tile_pool(name="accp", bufs=2, space="PSUM") as accp,
        ):
            idx_t = res.tile([P, nchunk * 8], i16)
            head_t = res.tile([P, nchunk], f16)
            iota_t = res.tile([P, P], f16)
            w4_t = res.tile([P, 4 * C], f32)
            ostage = res.tile([P, TILES * C], f32)

            nc.sync.dma_start(idx_t[:], idx_d[:])
            nc.sync.dma_start(head_t[:], head_d[:])
            nc.sync.dma_start(iota_t[:], iota_d[:])
            nc.sync.dma_start(w4_t[:], w4_d[:])

            import contextlib

            loop_cm = (
                tc.For_i(0, bench_loop, 1)
                if bench_loop > 0
                else contextlib.nullcontext()
            )
            with loop_cm:
              for g in range(ngrp):
                # gather the whole group, one call per source window
                g_t = gp.tile([P, gc * C], f32, tag="g")
                for w in range(WINS):
                    nidx = GROUP_T * capw * P
                    s0 = g * gc + GROUP_T * capw * w  # first gather slot
                    nc.gpsimd.dma_gather(
                        out_ap=g_t[
                            :, GROUP_T * capw * w * C : GROUP_T * capw * (w + 1) * C
                        ].rearrange("p (j c) -> p j c", c=C),
                        in_ap=emb_d[w * WSZ : min((w + 1) * WSZ, N_NODES), :],
                        idxs_ap=idx_t[:, s0 * 8 : s0 * 8 + nidx // 16],
                        num_idxs=nidx,
                        num_idxs_reg=nidx,
                        elem_size=C,
                        single_packet=False,
                    )

                # onehot slab for the group's chunks
                oh_t = ohp.tile([P, (gc // 4) * P], f32, tag="oh")
                nc.sync.dma_start(
                    oh_t[:], oh_d[:, (g * gc // 4) * P : ((g + 1) * gc // 4) * P]
                )

                for tloc in range(GROUP_T):
                    t = g * GROUP_T + tloc
                    j0 = t * cap  # first processing chunk of the tile

                    # head one-hots for all chunks of the tile (fp16, 2x DVE)
                    s_t = sp.tile([P, cap * P], f16, tag="s")
                    nc.vector.tensor_tensor(
                        out=s_t[:].rearrange("p (q n) -> p q n", n=P),
                        in0=head_t[:, j0 : j0 + cap].unsqueeze(2).to_broadcast(
                            [P, cap, P]
                        ),
                        in1=iota_t[:].unsqueeze(1).to_broadcast([P, cap, P]),
                        op=mybir.AluOpType.is_equal,
                    )

                    # Wsel for all chunks: one stacked matmul per 4 chunks
                    wsel_t = wselp.tile([P, cap * C], f32, tag="wsel")
                    for b in range(cap // 4):
                        gcol = ((j0 + 4 * b) // 4) * P - (g * gc // 4) * P
                        nc.tensor.matmul(
                            out=wsel_t[:, 4 * b * C : 4 * (b + 1) * C],
                            lhsT=oh_t[:, gcol : gcol + P],
                            rhs=w4_t[:],
                            start=True,
                            stop=True,
                        )

                    # V = G * Wsel for the whole tile in one DVE op.
                    # G slots for (w, kw) sit at group offset
                    # 7*capw*w + tloc*capw + kw -> strided 3D view.
                    v_t = vp.tile([P, cap * C], f16, tag="v")
                    g_view = _ap3(
                        g_t[:],
                        tloc * capw * C,
                        GROUP_T * capw * C,
                        WINS,
                        capw * C,
                    )
                    nc.vector.tensor_tensor(
                        out=v_t[:].rearrange("p (w x) -> p w x", w=WINS),
                        in0=g_view,
                        in1=wsel_t[:].rearrange("p (w x) -> p w x", w=WINS),
                        op=mybir.AluOpType.mult,
                    )

                    # scatter-accumulate the tile's chunks into PSUM
                    acc_t = accp.tile([P, C], f32, tag="acc")
                    for q in range(cap):
                        nc.tensor.matmul(
                            out=acc_t[:],
                            lhsT=s_t[:, q * P : (q + 1) * P],
                            rhs=v_t[:, q * C : (q + 1) * C],
                            start=(q == 0),
                            stop=(q == cap - 1),
                        )

                    nc.scalar.copy(
                        out=ostage[:, t * C : (t + 1) * C], in_=acc_t[:]
                    )

            nc.sync.dma_start(
                out_d[:].rearrange("(t p) c -> p t c", p=P),
                ostage[:].rearrange("p (t c) -> p t c", c=C),
            )

    nc.finalize()
    return nc


def _prep(all_emb, edge_index, edge_type, weight, aug_edge_weight):
    """Host-side binning/padding. Returns (capw, in_maps)."""
    head = np.asarray(edge_index[0], dtype=np.int64)
    tail = np.asarray(edge_index[1], dtype=np.int64)
    etype = np.asarray(edge_type, dtype=np.int64)
    aug = np.asarray(aug_edge_weight, dtype=np.float32).reshape(-1)
    emb = np.ascontiguousarray(np.asarray(all_emb, dtype=np.float32))
    w = np.asarray(weight, dtype=np.float32)

    order = np.argsort(head, kind="stable")
    h_s = head[order]
    bounds = np.searchsorted(h_s, np.arange(N_CORES + 1) * NPC)

    capw = 1
    per_core = []
    for c_i in range(N_CORES):
        e_idx = order[bounds[c_i] : bounds[c_i + 1]]
        h_loc = h_s[bounds[c_i] : bounds[c_i + 1]] - c_i * NPC
        t_loc = tail[e_idx]
        tw = (h_loc >> 7) * WINS + t_loc // WSZ  # (tile, window) bucket
        cnt = np.bincount(tw, minlength=TILES * WINS)
        capw = max(capw, int(-(-cnt.max() // P)))
        per_core.append((e_idx, h_loc, t_loc, tw, cnt))

    cap = WINS * capw
    nchunk = TILES * cap
    gc = GROUP_T * cap
    ncol = (nchunk // 4) * P

    iota = np.tile(np.arange(P, dtype=np.float16), (P, 1))
    w4 = np.zeros((P, 4 * C), dtype=np.float32)
    for s in range(4):
        w4[RP * s : RP * s + R, s * C : (s + 1) * C] = w

    in_maps = []
    for c_i in range(N_CORES):
        e_idx, h_loc, t_loc, tw, cnt = per_core[c_i]
        o2 = np.argsort(tw, kind="stable")  # group edges by (tile, window)
        e_idx, h_loc, t_loc, tw = e_idx[o2], h_loc[o2], t_loc[o2], tw[o2]
        starts = np.cumsum(cnt) - cnt
        pos = np.arange(len(e_idx)) - starts[tw]

        tile_id = tw // WINS
        win = tw % WINS
        kw = pos >> 7
        p = pos & (P - 1)

        # processing chunk id (tile-major)
        j = tile_id * cap + win * capw + kw
        # gather slot (window-major within each 7-tile group)
        grp = tile_id // GROUP_T
        tloc = tile_id % GROUP_T
        slot = grp * gc + GROUP_T * capw * win + tloc * capw + kw
        gi = slot * P + p  # flat gather index

        idx16 = np.zeros((P, nchunk * 8), np.int16)
        val16 = (t_loc - win * WSZ).astype(np.int16)
        rows = (gi % 16).astype(np.int64)
        cols = (gi // 16).astype(np.int64)
        for rep in range(8):
            idx16[rep * 16 + rows, cols] = val16

        head_a = np.zeros((P, nchunk), dtype=np.float16)
        head_a[p, j] = (h_loc - (tile_id << 7)).astype(np.float16)

        oh = np.zeros((P, ncol), dtype=np.float32)
        q_r = RP * (j & 3) + etype[e_idx]
        col = (j >> 2) * P + p
        oh[q_r, col] = aug[e_idx]

        in_maps.append(
            {
                "all_emb": emb,
                "idx16": idx16,
                "head_local": head_a,
                "oh": oh,
                "w4": w4,
                "iota": iota,
            }
        )
    return capw, in_maps


def kernel(all_emb, edge_index, edge_type, weight, aug_edge_weight):
    capw, in_maps = _prep(all_emb, edge_index, edge_type, weight,
                          aug_edge_weight)
    if capw not in _NC_CACHE:
        _NC_CACHE[capw] = _build(capw)
    nc = _NC_CACHE[capw]

    trace = bool(int(os.environ.get("KERNEL_TRACE", "0")))
    res = run_bass_kernel_spmd(
        nc,
        in_maps,
        core_ids=list(range(N_CORES)),
        trace=trace,
    )
    kernel._last_results = res
    out = np.concatenate(
        [res.results[c_i]["out"][:NPC] for c_i in range(N_CORES)], axis=0
    )
    return out


# revision 16
# speedup vs baseline: 153.6341x; 2.3025x over previous
"""Trainium2 Bass kernel for nn_Aggregator (GNN relational message passing).

Computes: out[h] = sum_{e: head_e==h} all_emb[tail_e] * weight[type_e] * aug_e

Strategy (8 NeuronCores, SPMD):
  - Shard output nodes (head ranges of 12500) across the 8 cores; each core
    gets exactly the edges whose head falls in its range (host bins them).
    No collective needed - host concatenates the 8 output shards.
  - Host sorts edges by head into 128-node output tiles. Within a tile,
    edges are grouped by tail window (4 windows of 25000 source rows, since
    dma_gather indices are int16) and each (tile, window) run is padded to a
    fixed chunk capacity so the device schedule is fully static.
  - Gather: one dma_gather per (7-tile group, window) pulls all_emb rows
    (256B each) into SBUF, window-major. Processing is tile-major and reads
    the gather buffer through a strided AP.
  - Per 128-edge chunk:
      Wsel = onehotAugT^T @ W4stack on PE      -> aug_e*weight[type_e] [e,64]
      S    = is_equal(iota, head_local) on DVE -> head one-hot (fp16) [e,128]
      V    = G * Wsel on DVE (fp16 out)                              [e,64]
      acc(psum f32) += S^T @ V on PE   (accumulated per node tile)
    Tile flush: PSUM -> SBUF staging on ScalarE; one output DMA at the end.
"""

import os

import numpy as np

import concourse.bacc as bacc
import concourse.tile as tile
from concourse import bass, mybir
from concourse.bass_utils import run_bass_kernel_spmd

P = 128
C = 64  # channels
R = 24  # relations
RP = 32  # relations padded (4 chunks of 32 rows stack into 128 partitions)
N_NODES = 100000
N_CORES = 8
NPC = N_NODES // N_CORES  # 12500 nodes per core
TILES = (NPC + P - 1) // P  # 98 output tiles per core
WINS = 4
WSZ = 25000  # window size (int16 gather index reach)
GROUP_T = 7  # tiles per gather group (98 = 14 * 7)
NGRP = TILES // GROUP_T

_NC_CACHE = {}


def _ap3(ap, offset_elems, mid_step, mid_n, inner_n):
    """[128, mid_n, inner_n] view of a 2D SBUF tile AP with custom strides."""
    return bass.AP(
        ap.tensor,
        ap.offset + offset_elems,
        [list(ap.ap[0]), [mid_step, mid_n], [1, inner_n]],
    )


def _build(capw: int, ngrp: int = NGRP, bench_loop: int = 0):
    """Build the Bass module for per-(tile,window) chunk capacity capw.

    bench_loop > 0 wraps the whole compute schedule in a device-side For_i
    that repeats it that many times (benchmark amplification only).
    """
    cap = WINS * capw  # chunks per tile
    nchunk = TILES * cap
    gc = GROUP_T * cap  # chunks per gather group
    ncol = (nchunk // 4) * P  # onehot columns

    nc = bacc.Bacc("TRN2", target_bir_lowering=False, num_swdge_queues=4)

    f32 = mybir.dt.float32
    f16 = mybir.dt.float16
    i16 = mybir.dt.int16

    emb_d = nc.dram_tensor("all_emb", [N_NODES, C], f32, kind="ExternalInput")
    idx_d = nc.dram_tensor("idx16", [P, nchunk * 8], i16, kind="ExternalInput")
    head_d = nc.dram_tensor("head_local", [P, nchunk], f16,
                            kind="ExternalInput")
    oh_d = nc.dram_tensor("oh", [P, ncol], f32, kind="ExternalInput")
    w4_d = nc.dram_tensor("w4", [P, 4 * C], f32, kind="ExternalInput")
    iota_d = nc.dram_tensor("iota", [P, P], f16, kind="ExternalInput")
    out_d = nc.dram_tensor("out", [TILES * P, C], f32, kind="ExternalOutput")

    with tile.TileContext(nc) as tc:
        with (
            tc.tile_pool(name="res", bufs=1) as res,
            tc.tile_pool(name="gp", bufs=2) as gp,
            tc.tile_pool(name="ohp", bufs=2) as ohp,
            tc.tile_pool(name="sp", bufs=2) as sp,
            tc.tile_pool(name="vp", bufs=2) as vp,
            tc.tile_pool(name="wselp", bufs=2, space="PSUM") as wselp,
            tc.tile_pool(name="accp", bufs=2, space="PSUM") as accp,
        ):
            idx_t = res.tile([P, nchunk * 8], i16)
            head_t = res.tile([P, nchunk], f16)
            iota_t = res.tile([P, P], f16)
            w4_t = res.tile([P, 4 * C], f32)
            ostage = res.tile([P, TILES * C], f32)

            nc.sync.dma_start(idx_t[:], idx_d[:])
            nc.sync.dma_start(head_t[:], head_d[:])
            nc.sync.dma_start(iota_t[:], iota_d[:])
            nc.sync.dma_start(w4_t[:], w4_d[:])

            import contextlib

            loop_cm = (
                tc.For_i(0, bench_loop, 1)
                if bench_loop > 0
                else contextlib.nullcontext()
            )
            with loop_cm:
              for g in range(ngrp):
                # gather the whole group, one call per source window
                g_t = gp.tile([P, gc * C], f32, tag="g")
                for w in range(WINS):
                    nidx = GROUP_T * capw * P
                    s0 = g * gc + GROUP_T * capw * w  # first gather slot
                    nc.gpsimd.dma_gather(
                        out_ap=g_t[
                            :, GROUP_T * capw * w * C : GROUP_T * capw * (w + 1) * C
                        ].rearrange("p (j c) -> p j c", c=C),
                        in_ap=emb_d[w * WSZ : min((w + 1) * WSZ, N_NODES), :],
                        idxs_ap=idx_t[:, s0 * 8 : s0 * 8 + nidx // 16],
                        num_idxs=nidx,
                        num_idxs_reg=nidx,
                        elem_size=C,
                        single_packet=False,
                        queue_num=w,
                    )

                # onehot slab for the group's chunks
                oh_t = ohp.tile([P, (gc // 4) * P], f32, tag="oh")
                nc.sync.dma_start(
                    oh_t[:], oh_d[:, (g * gc // 4) * P : ((g + 1) * gc // 4) * P]
                )

                for tloc in range(GROUP_T):
                    t = g * GROUP_T + tloc
                    j0 = t * cap  # first processing chunk of the tile

                    # head one-hots for all chunks of the tile (fp16, 2x DVE)
                    s_t = sp.tile([P, cap * P], f16, tag="s")
                    nc.vector.tensor_tensor(
                        out=s_t[:].rearrange("p (q n) -> p q n", n=P),
                        in0=head_t[:, j0 : j0 + cap].unsqueeze(2).to_broadcast(
                            [P, cap, P]
                        ),
                        in1=iota_t[:].unsqueeze(1).to_broadcast([P, cap, P]),
                        op=mybir.AluOpType.is_equal,
                    )

                    # Wsel for all chunks: one stacked matmul per 4 chunks
                    wsel_t = wselp.tile([P, cap * C], f32, tag="wsel")
                    for b in range(cap // 4):
                        gcol = ((j0 + 4 * b) // 4) * P - (g * gc // 4) * P
                        nc.tensor.matmul(
                            out=wsel_t[:, 4 * b * C : 4 * (b + 1) * C],
                            lhsT=oh_t[:, gcol : gcol + P],
                            rhs=w4_t[:],
                            start=True,
                            stop=True,
                        )

                    # V = G * Wsel for the whole tile in one DVE op.
                    # G slots for (w, kw) sit at group offset
                    # 7*capw*w + tloc*capw + kw -> strided 3D view.
                    v_t = vp.tile([P, cap * C], f16, tag="v")
                    g_view = _ap3(
                        g_t[:],
                        tloc * capw * C,
                        GROUP_T * capw * C,
                        WINS,
                        capw * C,
                    )
                    nc.vector.tensor_tensor(
                        out=v_t[:].rearrange("p (w x) -> p w x", w=WINS),
                        in0=g_view,
                        in1=wsel_t[:].rearrange("p (w x) -> p w x", w=WINS),
                        op=mybir.AluOpType.mult,
                    )

                    # scatter-accumulate the tile's chunks into PSUM
                    acc_t = accp.tile([P, C], f32, tag="acc")
                    for q in range(cap):
                        nc.tensor.matmul(
                            out=acc_t[:],
                            lhsT=s_t[:, q * P : (q + 1) * P],
                            rhs=v_t[:, q * C : (q + 1) * C],
                            start=(q == 0),
                            stop=(q == cap - 1),
                        )

                    nc.scalar.copy(
                        out=ostage[:, t * C : (t + 1) * C], in_=acc_t[:]
                    )

            nc.sync.dma_start(
                out_d[:].rearrange("(t p) c -> p t c", p=P),
                ostage[:].rearrange("p (t c) -> p t c", c=C),
            )

    nc.finalize()
    return nc


def _prep(all_emb, edge_index, edge_type, weight, aug_edge_weight):
    """Host-side binning/padding. Returns (capw, in_maps)."""
    head = np.asarray(edge_index[0], dtype=np.int64)
    tail = np.asarray(edge_index[1], dtype=np.int64)
    etype = np.asarray(edge_type, dtype=np.int64)
    aug = np.asarray(aug_edge_weight, dtype=np.float32).reshape(-1)
    emb = np.ascontiguousarray(np.asarray(all_emb, dtype=np.float32))
    w = np.asarray(weight, dtype=np.float32)

    order = np.argsort(head, kind="stable")
    h_s = head[order]
    bounds = np.searchsorted(h_s, np.arange(N_CORES + 1) * NPC)

    capw = 1
    per_core = []
    for c_i in range(N_CORES):
        e_idx = order[bounds[c_i] : bounds[c_i + 1]]
        h_loc = h_s[bounds[c_i] : bounds[c_i + 1]] - c_i * NPC
        t_loc = tail[e_idx]
        tw = (h_loc >> 7) * WINS + t_loc // WSZ  # (tile, window) bucket
        cnt = np.bincount(tw, minlength=TILES * WINS)
        capw = max(capw, int(-(-cnt.max() // P)))
        per_core.append((e_idx, h_loc, t_loc, tw, cnt))

    cap = WINS * capw
    nchunk = TILES * cap
    gc = GROUP_T * cap
    ncol = (nchunk // 4) * P

    iota = np.tile(np.arange(P, dtype=np.float16), (P, 1))
    w4 = np.zeros((P, 4 * C), dtype=np.float32)
    for s in range(4):
        w4[RP * s : RP * s + R, s * C : (s + 1) * C] = w

    in_maps = []
    for c_i in range(N_CORES):
        e_idx, h_loc, t_loc, tw, cnt = per_core[c_i]
        o2 = np.argsort(tw, kind="stable")  # group edges by (tile, window)
        e_idx, h_loc, t_loc, tw = e_idx[o2], h_loc[o2], t_loc[o2], tw[o2]
        starts = np.cumsum(cnt) - cnt
        pos = np.arange(len(e_idx)) - starts[tw]

        tile_id = tw // WINS
        win = tw % WINS
        kw = pos >> 7
        p = pos & (P - 1)

        # processing chunk id (tile-major)
        j = tile_id * cap + win * capw + kw
        # gather slot (window-major within each 7-tile group)
        grp = tile_id // GROUP_T
        tloc = tile_id % GROUP_T
        slot = grp * gc + GROUP_T * capw * win + tloc * capw + kw
        gi = slot * P + p  # flat gather index

        idx16 = np.zeros((P, nchunk * 8), np.int16)
        val16 = (t_loc - win * WSZ).astype(np.int16)
        rows = (gi % 16).astype(np.int64)
        cols = (gi // 16).astype(np.int64)
        for rep in range(8):
            idx16[rep * 16 + rows, cols] = val16

        head_a = np.zeros((P, nchunk), dtype=np.float16)
        head_a[p, j] = (h_loc - (tile_id << 7)).astype(np.float16)

        oh = np.zeros((P, ncol), dtype=np.float32)
        q_r = RP * (j & 3) + etype[e_idx]
        col = (j >> 2) * P + p
        oh[q_r, col] = aug[e_idx]

        in_maps.append(
            {
                "all_emb": emb,
                "idx16": idx16,
                "head_local": head_a,
                "oh": oh,
                "w4": w4,
                "iota": iota,
            }
        )
    return capw, in_maps


def kernel(all_emb, edge_index, edge_type, weight, aug_edge_weight):
    capw, in_maps = _prep(all_emb, edge_index, edge_type, weight,
                          aug_edge_weight)
    if capw not in _NC_CACHE:
        _NC_CACHE[capw] = _build(capw)
    nc = _NC_CACHE[capw]

    trace = bool(int(os.environ.get("KERNEL_TRACE", "0")))
    res = run_bass_kernel_spmd(
        nc,
        in_maps,
        core_ids=list(range(N_CORES)),
        trace=trace,
    )
    kernel._last_results = res
    out = np.concatenate(
        [res.results[c_i]["out"][:NPC] for c_i in range(N_CORES)], axis=0
    )
    return out
